# revision 21
# baseline (speedup 1.0000x reference)
"""Trainium2 Bass kernel for the EulerFlexRNNCell SDE scan.

Math (per Euler step, complex state x [B,10], f = [Re x, Im x, params]):
  a    = x @ A.T + mlp_ar(f) + 1j*mlp_ai(f)
  bc   = mlp_br(f) + 1j*mlp_bi(f)
  bmat = einsum('kdm,bk->bdm', Bten, x) + bc.reshape(B,10,2)
  xn   = x + a*DT + einsum('bdm,bm->bd', bmat, dW)

Device formulation (per core: 128 trajectories, 2 pipelined groups of 64,
feature-major layout [rows, batch]):
  State S [121, Bg] fp16 rows (block starts obey the SBUF start-partition
  rule {0,32,64,96}):
    0:20   u    = (Re/Im bmat_m0) * dW0        20:40  v = (bmat_m1) * dW1
    40:64  zeros                               64:84  y_hi = fp16(x + a*DT)
    84:96  zeros                               96:116 y_lo = y_hi residual
    116:120 params                             120    ones
  so x = u + v + y_hi + y_lo, re-summed implicitly inside the next step's
  matmuls via 4x-replicated weight rows (contraction K=109 <= 128 is free).

  L1: 8 matmuls  h[:,64c:64c+64] = W1chunk[109,128]^T @ S      (PSUM f32)
  tanh: one ACT op [128,512] PSUM->SBUF fp16
  L2 into o2 [84, Bg] PSUM: rows 0:40 = bmat blocks (Re m0, Im m0, Re m1,
    Im m1), rows 64:84 = x/DT + a (the 1/DT identity is folded into the
    combined linear matmul; biases ride on the ones row):
      1 matmul  LIN[109,84]   (A/Bten linear terms + 1/DT identity + biases)
      4 matmuls Wa2c[128,20] -> o2[64:84]   (block-diag trimmed)
      4 matmuls Wb2c[128,40] -> o2[0:40]
  Update (2 DVE ops, writing next S):
      S'[0:84]   = o2[0:84] * SC[:, t]    (SC rows: dW0 x20, dW1 x20, 0 x24,
                                           DT x20  -> u, v, zeros, y_hi)
      S'[96:116] = o2[64:84]*DT - S'[64:84]          (y_lo, exact residual)
  S'[0:84] is DMA'd out each step (from the vector engine's SWDGE so the
  data dependency is free by program order; walrus allows only 1 sync wait
  per DMA); the host sums u+v+y_hi (y_lo, a <=2.4e-4 correction, stays
  on-device only), prepends x0, and rebuilds the Hermitian rho trajectory
  in float64.
"""

import os
import sys

for _p in ("/opt/trn_rl_repo", "/opt/pypackages"):
    if os.path.isdir(_p) and _p not in sys.path:
        sys.path.append(_p)

import numpy as np
import bass_rust
from concourse import bass, tile, mybir
from concourse.bass_utils import run_bass_kernel_spmd

PDIM = 4
D = 10
NOISE_M = 2
H = 256
B = 1024
DT = 1.0 / 128.0
NSTEPS = 127
NCORES = 8
BC = B // NCORES       # trajectories per core
BG = BC // 2           # trajectories per group
HALL = 4 * H           # 1024
SROWS = 121
F16 = mybir.dt.float16
IU = np.triu_indices(PDIM)

_NC_CACHE = {}


def build_nc(nsteps=NSTEPS):
    nc = bass.Bass()
    s0_d = nc.declare_dram_parameter("s0", [SROWS, BC], F16, isOutput=False)
    w1_d = nc.declare_dram_parameter("w1", [SROWS, HALL], F16, isOutput=False)
    wa2_d = nc.declare_dram_parameter("wa2", [128, 80], F16, isOutput=False)
    wb2_d = nc.declare_dram_parameter("wb2", [128, 160], F16, isOutput=False)
    lin_d = nc.declare_dram_parameter("lin", [SROWS, 84], F16, isOutput=False)
    sc_d = nc.declare_dram_parameter("sc", [84, nsteps * BC], F16, isOutput=False)
    pr_d = nc.declare_dram_parameter("pr", [2, 57, (nsteps + 1) * BG], F16,
                                     isOutput=False)
    yuv_d = nc.declare_dram_parameter("yuv", [2, 84, nsteps, BG], F16, isOutput=True)

    TCOLS = nsteps * BC
    NBLK = nsteps + 1          # state column-blocks (x0..x127)
    with tile.TileContext(nc) as tc:
        with (
            tc.tile_pool(name="const", bufs=1) as cpool,
            tc.tile_pool(name="work", bufs=1) as wpool,
            tc.tile_pool(name="psum", bufs=1, space=bass.MemorySpace.PSUM) as ppool,
        ):
            w1 = cpool.tile([SROWS, HALL], F16, tag="w1")
            wa2 = cpool.tile([128, 80], F16, tag="wa2")
            wb2 = cpool.tile([128, 160], F16, tag="wb2")
            lin = cpool.tile([SROWS, 84], F16, tag="lin")
            sc = cpool.tile([84, TCOLS], F16, tag="sc")

            # per-group state history: column block t = state after t steps
            sbig = [wpool.tile([SROWS, NBLK * BG], F16, name=f"sbig{g}",
                               tag=f"sbig{g}") for g in range(2)]
            s0stage = wpool.tile([SROWS, BC], F16, name="s0stage",
                                 tag="s0stage")
            gscratch = wpool.tile([1, 16], F16, name="gscratch",
                                  tag="gscratch")
            hs = [[wpool.tile([128, 8 * BG], F16, name=f"hs{g}{p}",
                               tag=f"hs{g}{p}") for p in range(2)]
                  for g in range(2)]
            h_ps = [[ppool.tile([128, 8 * BG], mybir.dt.float32,
                                name=f"h{g}{p}", tag=f"h{g}{p}")
                     for p in range(2)] for g in range(2)]
            o2_ps = [ppool.tile([84, BG], mybir.dt.float32, name=f"o2{g}",
                                tag=f"o2{g}") for g in range(2)]
            pscr = ppool.tile([2, 2], mybir.dt.float32, name="pscr",
                              tag="pscr")

            # --- init loads ---
            nc.sync.dma_start(out=w1[:, 0:512], in_=w1_d[:, 0:512])
            nc.sync.dma_start(out=w1[:, 512:1024], in_=w1_d[:, 512:1024])
            nc.sync.dma_start(out=wa2[:], in_=wa2_d[:])
            nc.sync.dma_start(out=wb2[:], in_=wb2_d[:])
            nc.sync.dma_start(out=lin[:], in_=lin_d[:])
            nchunk = 8 if nsteps >= 8 else 1
            ccols = TCOLS // nchunk
            for c in range(nchunk):
                cs = slice(c * ccols, (c + 1) * ccols)
                nc.sync.dma_start(out=sc[:, cs], in_=sc_d[:, cs])
            nc.sync.dma_start(out=s0stage[:], in_=s0_d[:])
            for g in range(2):
                # rows 64:121 (y_hi/pad/y_lo zeros + params + ones), all blocks
                nc.sync.dma_start(out=sbig[g][64:121, :], in_=pr_d[g])
            # TensorCopy instructions allow only ONE sync wait in walrus
            # codegen; absorb each init-DMA tick on DVE with its own 1-wait
            # in-place touch (self-copy) before any multi-dependency copy
            # needs them.
            nc.vector.tensor_copy(s0stage[0:1, 0:1], s0stage[0:1, 0:1])
            for g in range(2):
                nc.vector.tensor_copy(sbig[g][96:97, 0:1], sbig[g][96:97, 0:1])
            for g in range(2):
                nc.vector.tensor_copy(sbig[g][:, 0:BG],
                                      s0stage[:, g * BG:(g + 1) * BG])
            # absorb sc chunk-0 DMA tick on DVE
            nc.vector.tensor_copy(sc[0:1, 0:1], sc[0:1, 0:1])
            # absorb the pr-DMA ticks on the PE engine (the first matmuls read
            # sbig rows 64:121; Matmult allows a single sync wait) via tiny
            # dummy matmuls into a scratch PSUM tile
            for g in range(2):
                nc.tensor.matmul(pscr[0:2, 0:2],
                                 sbig[g][64:66, BG:BG + 2],
                                 sbig[g][64:66, BG + 2:BG + 4],
                                 start=True, stop=True)

            # --- the scan ---
            OCH = 16           # output drain chunk, in steps
            for t in range(nsteps):
                # absorb the next sc chunk's DMA tick on DVE well before use
                if nchunk > 1 and t % 16 == 8 and (t + 8) // 16 < nchunk:
                    c = (t + 8) // 16
                    nc.vector.tensor_copy(sc[0:1, c * ccols:c * ccols + 1],
                                          sc[0:1, c * ccols:c * ccols + 1])
                for g in range(2):
                    cur = sbig[g][:, t * BG:(t + 1) * BG]
                    h = h_ps[g][t % 2]
                    hst = hs[g][t % 2]
                    o2 = o2_ps[g]
                    for c in range(8):
                        nc.tensor.matmul(
                            h[:, c * BG:(c + 1) * BG],
                            w1[:, c * 128:(c + 1) * 128],
                            cur,
                            start=True, stop=True,
                        )
                    nc.scalar.activation(hst[:], h[:],
                                         mybir.ActivationFunctionType.Tanh)
                    nc.tensor.matmul(o2[0:84, :], lin[:], cur,
                                     start=True, stop=False,
                                     skip_group_check=True)
                    for c in range(4):
                        nc.tensor.matmul(
                            o2[64:84, :],
                            wa2[:, c * 20:(c + 1) * 20],
                            hst[:, c * BG:(c + 1) * BG],
                            start=False, stop=False, skip_group_check=True,
                        )
                    for c in range(4):
                        nc.tensor.matmul(
                            o2[0:40, :],
                            wb2[:, c * 40:(c + 1) * 40],
                            hst[:, (4 + c) * BG:(5 + c) * BG],
                            start=False, stop=(c == 3), skip_group_check=True,
                        )
                    nb = (t + 1) * BG  # next state block
                    off = t * BC + g * BG
                    nc.vector.tensor_tensor(
                        sbig[g][0:84, nb:nb + BG], o2[0:84, :],
                        sc[:, off:off + BG], mybir.AluOpType.mult,
                    )
                    nc.vector.scalar_tensor_tensor(
                        sbig[g][96:116, nb:nb + BG], o2[64:84, :], DT,
                        sbig[g][64:84, nb:nb + BG],
                        mybir.AluOpType.mult, mybir.AluOpType.subtract,
                    )
                # drain finished output chunks from gpsimd (touch absorbs the
                # DVE tick so the pool-queue DMA itself needs no data wait)
                if (t + 1) % OCH == 0 or t == nsteps - 1:
                    k = t // OCH
                    b0 = k * OCH + 1           # first block of chunk
                    bn = t + 2                 # one past last block
                    for g in range(2):
                        nc.gpsimd.tensor_copy(
                            gscratch[0:1, g * 8 + (k % 8):g * 8 + (k % 8) + 1],
                            sbig[g][0:1, bn * BG - 1:bn * BG])
                        nc.gpsimd.dma_start(
                            out=yuv_d[g, :, b0 - 1:bn - 1, :],
                            in_=sbig[g][0:84, b0 * BG:bn * BG])
    return nc


def _legalize_sync_waits(nc):
    """walrus codegen allows only ONE sync wait on most instruction structs
    (TensorTensor takes two). Two legalizations:

    1. Activations: Tile emits a redundant same-engine WAW wait (its
       optimize_sems elision pass is disabled); ACT executes its stream in
       order, so drop the self-wait when a cross-engine wait is present.
    2. The kernel-tail Drain carries a wait per touched semaphore (~20
       here); split it into a chain of single-wait drains.
    """
    from concourse import mybir
    nsplit = 0
    for f in nc.m.functions:
        for b in f.blocks:
            newlist = None
            insts = b.instructions
            for idx, inst in enumerate(insts):
                tname = type(inst).__name__
                si = inst.sync_info
                if si is None or len(si.on_wait) <= 1:
                    continue
                if tname == "InstActivation":
                    mine = inst.engine.name + "_"
                    kept = [w for w in si.on_wait
                            if not w.ant_name.startswith(mine)]
                    if len(kept) < len(si.on_wait):
                        inst.sync_info = bass_rust.SyncInfo(
                            on_wait=kept, on_update=si.on_update)
                elif tname == "InstDrain":
                    if newlist is None:
                        newlist = list(insts)
                    pre = []
                    for w in si.on_wait[:-1]:
                        nd = mybir.InstDrain(name=f"I-drainsplit{nsplit}",
                                             ins=[], outs=[])
                        nsplit += 1
                        nd.engine = inst.engine
                        nd.sync_info = bass_rust.SyncInfo(on_wait=[w],
                                                          on_update=[])
                        pre.append(nd)
                    inst.sync_info = bass_rust.SyncInfo(
                        on_wait=[si.on_wait[-1]], on_update=si.on_update)
                    pos = newlist.index(inst)
                    newlist[pos:pos] = pre
            if newlist is not None:
                b.instructions = newlist


# ---------------------------------------------------------------------------
# host-side prep
# ---------------------------------------------------------------------------

def _col_layout():
    """W1_aug column -> (mlp, h) mapping; chunks 0-3 interleave a_r/a_i,
    chunks 4-7 interleave b_r/b_i (64+64 rows each)."""
    colmap = []
    for c in range(8):
        m0, m1 = ("ar", "ai") if c < 4 else ("br", "bi")
        h0 = 64 * (c % 4)
        colmap += [(m0, h0 + j) for j in range(64)]
        colmap += [(m1, h0 + j) for j in range(64)]
    return colmap


def _expand_rows(x20, pmap=None, ones=None, ncols=None):
    """Place a 20-row x-map at row blocks {0,20,64,96}; params at 116:120,
    ones row at 120."""
    ncols = x20.shape[1] if ncols is None else ncols
    out = np.zeros((SROWS, ncols), dtype=np.float64)
    for r0 in (0, 20, 64, 96):
        out[r0:r0 + 20] = x20
    if pmap is not None:
        out[116:120] = pmap
    if ones is not None:
        out[120] = ones
    return out


def _host_weights(inp):
    f = lambda k: np.asarray(inp[k], dtype=np.float64)
    W1s = {"ar": f("Wa1r"), "ai": f("Wa1i"), "br": f("Wb1r"), "bi": f("Wb1i")}
    b1s = {"ar": f("ba1r"), "ai": f("ba1i"), "br": f("bb1r"), "bi": f("bb1i")}
    colmap = _col_layout()

    x20 = np.zeros((20, HALL))
    pmap = np.zeros((4, HALL))
    ones = np.zeros(HALL)
    for col, (mlp, h) in enumerate(colmap):
        x20[:, col] = W1s[mlp][0:20, h]
        pmap[:, col] = W1s[mlp][20:24, h]
        ones[col] = b1s[mlp][h]
    w1 = _expand_rows(x20, pmap, ones)

    Wa2r, Wa2i = f("Wa2r"), f("Wa2i")
    Wb2r, Wb2i = f("Wb2r"), f("Wb2i")
    wa2 = np.zeros((128, 80))
    wb2 = np.zeros((128, 160))
    for c in range(4):
        hsl = slice(64 * c, 64 * c + 64)
        wa2[0:64, 20 * c:20 * c + 10] = Wa2r[hsl, :]
        wa2[64:128, 20 * c + 10:20 * c + 20] = Wa2i[hsl, :]
        for d in range(10):
            wb2[0:64, 40 * c + d] = Wb2r[hsl, d * 2 + 0]
            wb2[64:128, 40 * c + 10 + d] = Wb2i[hsl, d * 2 + 0]
            wb2[0:64, 40 * c + 20 + d] = Wb2r[hsl, d * 2 + 1]
            wb2[64:128, 40 * c + 30 + d] = Wb2i[hsl, d * 2 + 1]

    A = np.asarray(inp["A"], dtype=np.complex128)
    Bten = np.asarray(inp["Bten"], dtype=np.complex128)
    Ar, Ai, Br, Bi = A.real, A.imag, Bten.real, Bten.imag
    lx = np.zeros((20, 84))
    for d in range(10):
        for m in range(2):
            lx[0:10, 20 * m + d] = Br[:, d, m]
            lx[10:20, 20 * m + d] = -Bi[:, d, m]
            lx[0:10, 20 * m + 10 + d] = Bi[:, d, m]
            lx[10:20, 20 * m + 10 + d] = Br[:, d, m]
        lx[0:10, 64 + d] = Ar[d, :]
        lx[10:20, 64 + d] = -Ai[d, :]
        lx[0:10, 74 + d] = Ai[d, :]
        lx[10:20, 74 + d] = Ar[d, :]
    lx[np.arange(10), 64 + np.arange(10)] += 1.0 / DT
    lx[10 + np.arange(10), 74 + np.arange(10)] += 1.0 / DT
    lones = np.zeros(84)
    ba2r, ba2i, bb2r, bb2i = f("ba2r"), f("ba2i"), f("bb2r"), f("bb2i")
    for d in range(10):
        for m in range(2):
            lones[20 * m + d] = bb2r[d * 2 + m]
            lones[20 * m + 10 + d] = bb2i[d * 2 + m]
        lones[64 + d] = ba2r[d]
        lones[74 + d] = ba2i[d]
    lin = _expand_rows(lx, None, lones, ncols=84)

    c16 = lambda x: np.ascontiguousarray(x, dtype=np.float16)
    return {"w1": c16(w1), "wa2": c16(wa2), "wb2": c16(wb2), "lin": c16(lin)}


def _host_percore(inp, core, nsteps=NSTEPS):
    rho = np.asarray(inp["rho"], dtype=np.complex128)
    params = np.asarray(inp["params"], dtype=np.float64)
    wvec = np.asarray(inp["wvec"], dtype=np.float64)
    tsl = slice(core * BC, (core + 1) * BC)
    x0 = rho[tsl][:, IU[0], IU[1]]  # [BC, 10]

    s0 = np.zeros((SROWS, BC), dtype=np.float16)
    xre_hi = x0.real.T.astype(np.float16)
    xim_hi = x0.imag.T.astype(np.float16)
    s0[64:74] = xre_hi
    s0[74:84] = xim_hi
    s0[96:106] = (x0.real.T - xre_hi.astype(np.float64)).astype(np.float16)
    s0[106:116] = (x0.imag.T - xim_hi.astype(np.float64)).astype(np.float16)
    s0[116:120] = params[tsl].T.astype(np.float16)
    s0[120] = 1.0

    sc = np.zeros((84, nsteps, BC), dtype=np.float16)
    w_c = wvec[tsl, :nsteps, :]  # [BC, nsteps, 2]
    sc[0:20] = w_c[:, :, 0].T[None, :, :].astype(np.float16)
    sc[20:40] = w_c[:, :, 1].T[None, :, :].astype(np.float16)
    sc[64:84] = np.float16(DT)

    nblk = nsteps + 1
    pr = np.zeros((2, 57, nblk * BG), dtype=np.float16)
    for g in range(2):
        pg = params[core * BC + g * BG:core * BC + (g + 1) * BG].T  # [4, BG]
        pr[g, 52:56] = np.tile(pg.astype(np.float16), (1, nblk))
        pr[g, 56] = 1.0
    return {"s0": s0, "sc": np.ascontiguousarray(sc.reshape(84, nsteps * BC)),
            "pr": pr}


def assemble_output(inp, yuvs, nsteps=NSTEPS):
    rho = np.asarray(inp["rho"], dtype=np.complex128)
    x0 = rho[:, IU[0], IU[1]]  # [B, 10]
    xs = np.empty((B, nsteps + 1, D), dtype=np.complex128)
    xs[:, 0] = x0
    for core in range(NCORES):
        Y = np.asarray(yuvs[core], dtype=np.float64)  # [2, 84, T, BG]
        xre = Y[:, 64:74] + Y[:, 0:10] + Y[:, 20:30]   # [2, 10, T, BG]
        xim = Y[:, 74:84] + Y[:, 10:20] + Y[:, 30:40]
        xt = (xre + 1j * xim).transpose(0, 3, 2, 1)    # [2, BG, T, 10]
        xs[core * BC:(core + 1) * BC, 1:] = xt.reshape(BC, NSTEPS, D)
    up = np.zeros((B, nsteps + 1, PDIM, PDIM), dtype=np.complex128)
    up[:, :, IU[0], IU[1]] = xs
    diag = np.einsum("...ii->...i", up)
    eye = np.eye(PDIM)
    out = up + np.conj(np.swapaxes(up, -1, -2)) - np.einsum(
        "...i,ij->...ij", np.conj(diag), eye)
    return out


def kernel(**inputs):
    if "nc" not in _NC_CACHE:
        nc = build_nc(NSTEPS)
        nc.finalize()
        _legalize_sync_waits(nc)
        _NC_CACHE["nc"] = nc
    nc = _NC_CACHE["nc"]
    shared = _host_weights(inputs)
    in_maps = []
    for core in range(NCORES):
        m = dict(shared)
        m.update(_host_percore(inputs, core))
        in_maps.append(m)
    res = run_bass_kernel_spmd(nc, in_maps, core_ids=list(range(NCORES)))
    yuvs = [np.asarray(res.results[c]["yuv"]).reshape(2, 84, NSTEPS, BG)
            for c in range(NCORES)]
    return assemble_output(inputs, yuvs)
